# revision 45
# baseline (speedup 1.0000x reference)
"""AdaIN (segment mean/std + EMA of style stats) distributed over 8 TRN2 NeuronCores.

Strategy — host-side segment bucketing + transposed layout:
  - The host deals each segment's rows evenly across the 8 cores into
    fixed-capacity buckets (content: CAP rows per (core, segment); style:
    SCAP), padding with zero rows. Pad rows contribute nothing to sums, and
    exact per-segment counts are computed on the host.
  - Data ships TRANSPOSED: partition p = half*64 + channel, free dim = rows.
    Each segment occupies a fixed, compile-time-constant column range, so the
    SPMD instruction stream is static.
  - pass 1 (per core): per segment, DMA the f32 tile, cast to a persistent
    bf16 SBUF cache (fused with accum_out => per-channel sum(x)), and an ACT
    Square pass with accum_out => sum(x^2). Style is reduced straight from
    the staged f32 tiles (no cache).
  - one 32KB AllReduce combines per-core partial sums (the module's own
    dist behavior); a tiny f32 matmul folds the two row-halves together.
  - stats math replicated on every core in [channel, segment] orientation:
    mean/std via host-provided 1/n and n/(n-1), the EMA across batch ids as
    tensor_tensor_scans along the segment axis, then a = g_std/c_std,
    b = g_mean - c_mean*a per (channel, segment).
  - pass 2 (per core): per segment, out = x*a + b where a,b are per-partition
    scalars -- one fused op per engine-span on DVE (tensor_scalar), ACT
    (activation Identity w/ scale+bias) and GPSIMD (tensor_scalar), from the
    bf16 cache; bf16 out DMA'd to HBM. Host undoes the permutation.
"""

import os
import sys

import numpy as np

for _p in ("/opt/trn_rl_repo",):
    if _p not in sys.path and os.path.isdir(_p):
        sys.path.insert(0, _p)

from concourse import bacc, bass, bass_utils, mybir, tile

F32 = mybir.dt.float32
BF16 = mybir.dt.bfloat16

N_CORES = 8
C = 64
B = 16
ALPHA = 0.1
EPS = 1e-8

# per-(core, segment) bucket capacities (rows; must be even)
CAP = 7936    # content: expected ~7813 +- 30 after even dealing
SCAP = 1984   # style:   expected ~1953 +- 15

# pass-2 engine column split of each CAP//2-wide segment tile
P2_DVE = 1600
P2_ACT = 1600

BISECT = "full"
RDMA_AR = False  # experimental remote_dma all-reduce: ~50us faster but hangs intermittently; keep off


def _chunks(total: int, step: int):
    t0 = 0
    while t0 < total:
        yield t0, min(step, total - t0)
        t0 += step


def build_nc(cap: int = CAP, scap: int = SCAP, n_cores: int = N_CORES):
    cs = cap // 2     # content cols per segment
    ss = scap // 2    # style cols per segment
    CC = B * cs       # content cols per core
    SC = B * ss

    nc = bacc.Bacc(
        "TRN2", target_bir_lowering=False, debug=False, num_devices=n_cores
    )
    xin = nc.dram_tensor("xin", [128, CC], F32, kind="ExternalInput")
    sin = nc.dram_tensor("sin", [128, SC], F32, kind="ExternalInput")
    hc = nc.dram_tensor("hc", [128, 4 * B], F32, kind="ExternalInput")
    pairp = nc.dram_tensor("pairp", [128, 128], F32, kind="ExternalInput")
    outb = nc.dram_tensor("outb", [128, CC], BF16, kind="ExternalOutput")

    ID = mybir.ActivationFunctionType.Identity
    SQ = mybir.ActivationFunctionType.Square

    with tile.TileContext(nc) as tc:
        with (
            tc.tile_pool(name="const", bufs=1) as constp,
            tc.tile_pool(name="dram", bufs=1, space="DRAM") as dramp,
        ):
            hc_sb = constp.tile([128, 4 * B], F32)
            nc.sync.dma_start(hc_sb[:], hc.ap())
            pairp_sb = constp.tile([128, 128], F32)
            nc.sync.dma_start(pairp_sb[:], pairp.ap())

            xc = constp.tile([128, CC], BF16)        # content cache
            Rv = constp.tile([128, 2 * B], F32)      # DVE accums [sx_c|sx_s]
            Rp = constp.tile([128, B], F32)          # ACT cast accums (sx_c part)
            Ra = constp.tile([128, 2 * B], F32)      # ACT accums [sx2_c|sx2_s]

            use_rdma = RDMA_AR and BISECT != "nocoll" and n_cores == 8
            if use_rdma:
                # recursive-doubling all-reduce state: clear sems + write the
                # three rounds' descriptors up front (data is read at trigger
                # time). No peer can send before its own pass-1 ends, so
                # clearing at kernel start is race-free.
                acc = constp.tile([128, 4 * B], F32)
                rbuf = constp.tile([128, 3, 4 * B], F32)
                nc.vector.memset(rbuf[:], 0.0)
                rsems = [nc.alloc_semaphore(f"rdar_r{r}") for r in range(3)]
                lsem = nc.alloc_semaphore("rdar_l")
                asem = nc.alloc_semaphore("rdar_a")
                with tc.tile_critical(name="rdar_prep"):
                    for sem in rsems + [lsem, asem]:
                        nc.gpsimd.sem_clear(sem)

            def all_reduce(dst_sb, srcs, tag, w=2 * B):
                inb = dramp.tile([128, w], F32, tag=f"arin_{tag}")
                outd = dramp.tile([128, w], F32, tag=f"arout_{tag}")
                for j, src in enumerate(srcs):
                    nc.sync.dma_start(inb[:, j * B : (j + 1) * B], src)
                if BISECT == "nocoll":
                    nc.sync.dma_start(outd[:], inb[:])
                else:
                    nc.gpsimd.collective_compute(
                        "AllReduce",
                        mybir.AluOpType.add,
                        replica_groups=[list(range(n_cores))],
                        ins=[inb.opt()],
                        outs=[outd.opt()],
                    )
                nc.sync.dma_start(dst_sb, outd[:])

            # ---------------- pass 1 (style interleaved into content) ------
            with (
                tc.tile_pool(name="p1s", bufs=2) as p1s,
                tc.tile_pool(name="p1c", bufs=3) as p1c,
                tc.tile_pool(name="p1d", bufs=2) as p1d,
            ):
                def style_seg(s):
                    st = p1s.tile([128, ss], F32, tag="st")
                    nc.sync.dma_start(st[:], sin.ap()[:, s * ss : (s + 1) * ss])
                    d1 = p1d.tile([128, ss], BF16, tag="sd1")
                    nc.scalar.activation(
                        d1[:], st[:], SQ,
                        accum_out=Ra[:, B + s : B + s + 1],
                    )
                    d2 = p1d.tile([128, ss], BF16, tag="sd2")
                    nc.vector.tensor_scalar(
                        d2[:], st[:], 1.0, 0.0, mybir.AluOpType.mult,
                        mybir.AluOpType.add,
                        accum_out=Rv[:, B + s : B + s + 1],
                    )

                cd = cs * 7 // 10  # DVE share of the content cast (rest: ACT)
                def content_seg(s):
                    ct = p1c.tile([128, cs], F32, tag="ct")
                    h = cs // 2
                    nc.sync.dma_start(ct[:, 0:h], xin.ap()[:, s * cs : s * cs + h])
                    nc.sync.dma_start(
                        ct[:, h:cs], xin.ap()[:, s * cs + h : (s + 1) * cs]
                    )
                    base = s * cs
                    nc.vector.tensor_scalar(
                        xc[:, base : base + cd], ct[:, 0:cd], 1.0, 0.0,
                        mybir.AluOpType.mult, mybir.AluOpType.add,
                        accum_out=Rv[:, s : s + 1],
                    )
                    nc.scalar.activation(
                        xc[:, base + cd : base + cs], ct[:, cd:cs],
                        mybir.ActivationFunctionType.Copy,
                        accum_out=Rp[:, s : s + 1],
                    )
                    d3 = p1d.tile([128, cs], BF16, tag="cd")
                    nc.scalar.activation(
                        d3[:], xc[:, base : base + cs], SQ,
                        accum_out=Ra[:, s : s + 1],
                    )

                for s in range(8):
                    content_seg(s)
                    style_seg(2 * s)
                    style_seg(2 * s + 1)
                for s in range(8, B):
                    content_seg(s)

            # ---------------- combined AllReduce ----------------
            if use_rdma:
                # assemble [sx_c | sx2_c | sx_s | sx2_s] into acc
                nc.vector.scalar_tensor_tensor(
                    acc[:, 0:B], Rv[:, 0:B], 1.0, Rp[:],
                    mybir.AluOpType.mult, mybir.AluOpType.add,
                )
                nc.vector.tensor_copy(acc[:, B : 2 * B], Ra[:, 0:B])
                nc.vector.tensor_copy(acc[:, 2 * B : 3 * B], Rv[:, B : 2 * B])
                nc.vector.tensor_copy(acc[:, 3 * B : 4 * B], Ra[:, B : 2 * B])
                with tc.tile_critical(name="rdar_run"):
                    nc.vector.nop(nofuse=True, hint="acc_ready").then_inc(
                        asem, 1
                    )
                    for r in range(3):
                        d = 1 << r
                        rdests = [None] * 8
                        rdests[d] = (0, d)
                        nc.gpsimd.wait_ge(asem, r + 1)  # acc has round-r value
                        nc.gpsimd.remote_dma_broadcast(
                            rbuf[:, r, :], acc[:], remote_sem=rsems[r],
                            local_sem=lsem, rdests=rdests,
                        )
                        nc.gpsimd.trigger_dma(count=None)
                        nc.vector.wait_ge(rsems[r], 2)         # partner data
                        nc.vector.wait_ge(lsem, 16 * (r + 1))  # send drained
                        nc.vector.tensor_tensor(
                            acc[:], acc[:], rbuf[:, r, :], mybir.AluOpType.add
                        ).then_inc(asem, 1)
                Rg_c = acc
            else:
                nc.vector.tensor_tensor(Rv[:, 0:B], Rv[:, 0:B], Rp[:],
                                        mybir.AluOpType.add)
                Rg_c = constp.tile([128, 4 * B], F32)
                all_reduce(
                    Rg_c[:],
                    [Rv[:, 0:B], Ra[:, 0:B], Rv[:, B : 2 * B], Ra[:, B : 2 * B]],
                    "c", w=4 * B,
                )

            # ------------- stats math ([channel, segment], replicated) ------
            # (the style chain below is issued first so it executes inside
            # the content-AllReduce wait window)
            rn_c, fac_c = hc_sb[:, 0:B], hc_sb[:, B : 2 * B]
            rn_s, fac_s = hc_sb[:, 2 * B : 3 * B], hc_sb[:, 3 * B : 4 * B]

            def fold(dst, src):
                # dst[p, j] = src[p%64, j] + src[64 + p%64, j]
                with tc.tile_pool(name="ps_fold", bufs=1, space="PSUM") as psf:
                    ps = psf.tile([128, 2 * B], F32, tag="fold")
                    nc.tensor.matmul(ps[:], pairp_sb[:], src, start=True,
                                     stop=True)
                    nc.vector.tensor_copy(dst, ps[:])

            def seg_stats(S2, rn, fac, mean_out, std_out):
                nc.vector.tensor_tensor(mean_out, S2[:, 0:B], rn,
                                        mybir.AluOpType.mult)
                ex2 = constp.tile([128, B], F32, tag="ts_ex2")
                nc.vector.tensor_tensor(ex2[:], S2[:, B : 2 * B], rn,
                                        mybir.AluOpType.mult)
                m2 = constp.tile([128, B], F32, tag="ts_m2")
                nc.scalar.square(m2[:], mean_out)
                var = constp.tile([128, B], F32, tag="ts_var")
                nc.vector.tensor_sub(var[:], ex2[:], m2[:])
                nc.vector.tensor_tensor(var[:], var[:], fac, mybir.AluOpType.mult)
                nc.vector.tensor_scalar_max(var[:], var[:], 0.0)
                nc.scalar.sqrt(std_out, var[:])
                nc.vector.tensor_scalar_add(std_out, std_out, EPS)

            S_s = constp.tile([128, 2 * B], F32)
            fold(S_s[:], Rg_c[:, 2 * B : 4 * B])
            mean_s = constp.tile([128, B], F32)
            std_s = constp.tile([128, B], F32)
            seg_stats(S_s, rn_s, fac_s, mean_s[:], std_s[:])

            # EMA along segments as scans: g_j = 0.9*g_{j-1} + w_j*s_j,
            # w_0 = 1 (globals start as batch 0's style stats), w_j = 0.1.
            # mean_s/std_s are replicated on both halves, so scan each
            # directly -- no partition moves needed anywhere.
            c09 = constp.tile([128, B], F32)
            nc.vector.memset(c09[:], 1.0 - ALPHA)

            def ema(src, tag):
                smk = constp.tile([128, B], F32, tag=f"ema_in_{tag}")
                nc.vector.tensor_scalar_mul(smk[:], src, ALPHA)
                nc.vector.tensor_copy(smk[:, 0:1], src[:, 0:1])
                gout = constp.tile([128, B], F32, tag=f"ema_out_{tag}")
                nc.vector.tensor_tensor_scan(
                    gout[:], c09[:], smk[:], 0.0,
                    mybir.AluOpType.mult, mybir.AluOpType.add,
                )
                return gout

            g_mean = ema(mean_s[:], "m")
            g_std = ema(std_s[:], "s")

            S_c = constp.tile([128, 2 * B], F32)
            fold(S_c[:], Rg_c[:, 0 : 2 * B])
            mean_c = constp.tile([128, B], F32)
            std_c = constp.tile([128, B], F32)
            seg_stats(S_c, rn_c, fac_c, mean_c[:], std_c[:])

            # a = g_std/std_c ; b = g_mean - mean_c*a  (all replicated)
            rstd = constp.tile([128, B], F32)
            nc.vector.reciprocal(rstd[:], std_c[:])
            coefA = constp.tile([128, B], F32)
            nc.vector.tensor_tensor(coefA[:], g_std[:], rstd[:],
                                    mybir.AluOpType.mult)
            amc = constp.tile([128, B], F32)
            nc.vector.tensor_tensor(amc[:], mean_c[:], coefA[:],
                                    mybir.AluOpType.mult)
            coefB = constp.tile([128, B], F32)
            nc.vector.tensor_sub(coefB[:], g_mean[:], amc[:])

            if BISECT == "nopass2":
                nc.sync.dma_start(outb.ap()[0:C, 0:B], coefA[0:C, :])
                nc.compile()
                return nc

            # ---------------- pass 2 ----------------
            d0, d1_ = P2_DVE, P2_DVE + P2_ACT
            with tc.tile_pool(name="p2o", bufs=3) as p2o:
                for s in range(B):
                    base = s * cs
                    ot = p2o.tile([128, cs], BF16, tag="ot")
                    sa = coefA[:, s : s + 1]
                    sb = coefB[:, s : s + 1]
                    nc.vector.tensor_scalar(
                        ot[:, 0:d0], xc[:, base : base + d0], sa, sb,
                        mybir.AluOpType.mult, mybir.AluOpType.add,
                    )
                    nc.scalar.activation(
                        ot[:, d0:d1_], xc[:, base + d0 : base + d1_], ID,
                        bias=sb, scale=sa,
                    )
                    nc.gpsimd.tensor_scalar(
                        ot[:, d1_:cs], xc[:, base + d1_ : base + cs], sa, sb,
                        mybir.AluOpType.mult, mybir.AluOpType.add,
                    )
                    nc.sync.dma_start(outb.ap()[:, base : base + cs], ot[:])

    nc.compile()
    return nc


_NC_CACHE = {}


def _get_nc(cap=CAP, scap=SCAP, n_cores=N_CORES):
    key = (cap, scap, n_cores)
    if key not in _NC_CACHE:
        _NC_CACHE[key] = build_nc(cap, scap, n_cores)
    return _NC_CACHE[key]


def _deal(idx: np.ndarray, cap: int, n_cores: int):
    """Deal each segment's rows evenly across cores into cap-sized buckets.

    Returns G[(core, seg, cap)] int64 row ids, with N (== len(idx)) marking
    pad slots, and the exact per-segment counts.
    """
    n = len(idx)
    order = np.argsort(idx, kind="stable")
    counts = np.bincount(idx, minlength=B)[:B]
    G = np.full((n_cores, B, cap), n, dtype=np.int64)
    off = 0
    for s in range(B):
        rows_s = order[off : off + counts[s]]
        off += counts[s]
        splits = (np.arange(n_cores + 1) * counts[s]) // n_cores
        for k in range(n_cores):
            ck = rows_s[splits[k] : splits[k + 1]]
            G[k, s, : len(ck)] = ck
    return G, counts


def _to_device_layout(feats: np.ndarray, G: np.ndarray, cap: int):
    """(N, 64) f32 + bucket map -> per-core [128, B*cap//2] f32 arrays."""
    n = feats.shape[0]
    fz = np.concatenate([feats, np.zeros((1, C), np.float32)], axis=0)
    res = []
    for k in range(G.shape[0]):
        Xk = fz[G[k].reshape(-1)]                      # (B*cap, 64)
        B4 = Xk.reshape(B, 2, cap // 2, C)             # (s, h, r, c)
        res.append(
            np.ascontiguousarray(
                B4.transpose(1, 3, 0, 2).reshape(128, B * (cap // 2))
            )
        )
    return res


def _host_inputs(content_feats, style_feats, content_batch_indices,
                 style_batch_indices, cap, scap):
    cfe = np.asarray(content_feats, np.float32)
    sfe = np.asarray(style_feats, np.float32)
    cidx = np.asarray(content_batch_indices, np.int64)
    sidx = np.asarray(style_batch_indices, np.int64)

    Gc, cnt_c = _deal(cidx, cap, N_CORES)
    Gs, cnt_s = _deal(sidx, scap, N_CORES)
    xins = _to_device_layout(cfe, Gc, cap)
    sins = _to_device_layout(sfe, Gs, scap)

    nc_ = np.maximum(cnt_c.astype(np.float64), 1.0)
    ns_ = np.maximum(cnt_s.astype(np.float64), 1.0)
    hrow = np.concatenate(
        [1.0 / nc_, nc_ / np.maximum(nc_ - 1.0, 1.0),
         1.0 / ns_, ns_ / np.maximum(ns_ - 1.0, 1.0)]
    ).astype(np.float32)
    hc = np.ascontiguousarray(np.tile(hrow[None, :], (128, 1)))
    p = np.arange(128)
    pairp = (p[:, None] % C == np.arange(128)[None, :] % C).astype(np.float32)
    in_maps = [
        {"xin": xins[k], "sin": sins[k], "hc": hc, "pairp": pairp}
        for k in range(N_CORES)
    ]
    return in_maps, Gc


def _assemble_output(results, Gc, cap, n_rows):
    out = np.zeros((n_rows, C), np.float32)
    for k in range(N_CORES):
        O = np.asarray(results[k]["outb"]).astype(np.float32)
        rows = (
            O.reshape(2, C, B, cap // 2)
            .transpose(2, 0, 3, 1)
            .reshape(B * cap, C)
        )
        gk = Gc[k].reshape(-1)
        mask = gk < n_rows
        out[gk[mask]] = rows[mask]
    return out


def _pick_caps(content_batch_indices, style_batch_indices):
    cidx = np.asarray(content_batch_indices, np.int64)
    sidx = np.asarray(style_batch_indices, np.int64)
    need_c = int(np.ceil(np.bincount(cidx, minlength=B)[:B].max() / N_CORES))
    need_s = int(np.ceil(np.bincount(sidx, minlength=B)[:B].max() / N_CORES))

    def rnd(x):
        return ((x + 63) // 64) * 64

    return max(CAP, rnd(need_c)), max(SCAP, rnd(need_s))


def kernel(
    content_feats: np.ndarray,
    style_feats: np.ndarray,
    content_batch_indices: np.ndarray,
    style_batch_indices: np.ndarray,
    num_batches=B,
) -> np.ndarray:
    n_c = content_feats.shape[0]
    cap, scap = _pick_caps(content_batch_indices, style_batch_indices)
    in_maps, Gc = _host_inputs(
        content_feats, style_feats, content_batch_indices,
        style_batch_indices, cap, scap,
    )
    nc = _get_nc(cap, scap)
    res = bass_utils.run_bass_kernel_spmd(nc, in_maps, core_ids=list(range(N_CORES)))
    return _assemble_output(res.results, Gc, cap, n_c)
